# revision 19
# baseline (speedup 1.0000x reference)
"""Distributed attention kernel for 8 TRN2 NeuronCores (v8, 294us HW).

Sharding: data-parallel over (batch, t-chunk). Core c handles batch c//4,
query rows (c%4)*512 .. +512. Each core computes full K/V for its batch
(duplicated across the 4 cores of a batch group), its own 512-query-row
slice of attention, and the out-projection for those rows. No collectives
(v4 tried a 4-core AllGather for K/V: correct, but the non-Shared 4-rank
gather took 81us plus ~60us of DMA in/out and cannot be hidden behind the
K->QK->exp dependency chain -> slower than recomputing). fp8 DoubleRow
projections (v3, 298us) fail the 2e-2 gate at 6.3e-2: e4m3's ~4%
quantization passes straight to the projection outputs (signal and noise
both grow as sqrt(K)) and the 9-sigma scores amplify it through exp.
All matmul operands are bf16, f32 PSUM.

Optimizations over the 339.5us v2 baseline (each verified on HW traces):
  * QK^T (contraction 64): the per-iteration even/odd head matmuls write
    halves of one shared PSUM tile, so all become schedulable at the same
    event and the Tile scheduler keeps them adjacent in program order;
    their SBUF base partitions 0/64 land in distinct PE row groups, so
    each e/o pair runs CONCURRENTLY (PE array row tiling) -> QK time
    halves (start deltas 3-6ns on HW).
  * Two [128,1024] score tiles per iteration (T-tile granular, [even|odd]
    layout): exp on tile a frees the next QK pair while exp on tile b
    runs -> ACT issues exps back-to-back (no QK-after-exp serialization;
    exp period 2.7us -> ~1.1us).
  * reciprocal_approx_fast + per-pair normalize: SUMT[m] is final at pair
    m's end, so the out-projection's m-chains never wait on a batched
    4-head reciprocal; pso/obuf deepened to 4 buffers.
  * DMA issue order = first-use order (wq/xq, xkv, wk, wv, masks, wo) so
    the V-phase matmuls aren't starved behind weights needed later.

Math: S.T = (K_h @ Q_h.T)/8 per head; P.T = exp(S.T) * M.T (no max-sub:
scores are ~N(0,1)); [summed.T_h ; denom] from a ones-augmented V in one
PV accumulation; normalize by 1/(denom+eps); denom=0 rows give summed=0
-> out = bo, matching the wipe.

Perf structure: K.T projection tiles 1..7 interleaved into the attention
pair loop to fill ACT-bound gaps and keep the PE HAM-warm. PSUM budget:
s_a+s_b (4 banks) + pv_e+pv_o (2) + interleaved kproj (2) = 8.

Engine occupancy at 294us: TensorE ~84% (the bottleneck: 218us of ideal
matmul cycles with this duplicated-K/V sharding), ACT dense through the
156us attention window, DVE ~57%.
"""

import sys

sys.path.insert(0, "/opt/trn_rl_repo")

import numpy as np

import concourse.bass as bass
import concourse.bacc as bacc
import concourse.mybir as mybir
import concourse.tile as tile
from concourse.bass_utils import run_bass_kernel_spmd

F32 = mybir.dt.float32
BF16 = mybir.dt.bfloat16

B, T, D = 2, 2048, 1024
H, HD = 16, 64
TC = 512
NCORES = 8
KD = D // 128   # 8 d-tiles
NT = T // 128   # 16 T-tiles
VW = H * (HD + 1)  # 1040 v_aug width
EXP_SCALE = 1.0 / np.sqrt(HD)


def build_nc():
    nc = bacc.Bacc(
        "TRN2",
        target_bir_lowering=False,
        debug=False,
        enable_asserts=False,
        num_devices=NCORES,
    )

    xqT = nc.dram_tensor("xqT", [D, TC], BF16, kind="ExternalInput").ap()
    xkvT = nc.dram_tensor("xkvT", [D, T], BF16, kind="ExternalInput").ap()
    maskT = nc.dram_tensor("maskT", [T, TC], BF16, kind="ExternalInput").ap()
    wqT = nc.dram_tensor("wqT", [D, D], BF16, kind="ExternalInput").ap()
    wkT = nc.dram_tensor("wkT", [D, D], BF16, kind="ExternalInput").ap()
    wvT = nc.dram_tensor("wvT", [D, D], BF16, kind="ExternalInput").ap()
    woT = nc.dram_tensor("woT", [D, D], BF16, kind="ExternalInput").ap()
    bo = nc.dram_tensor("bo", [1, D], F32, kind="ExternalInput").ap()
    out = nc.dram_tensor("out", [TC, D], F32, kind="ExternalOutput").ap()

    with tile.TileContext(nc) as tc:
        with (
            tc.tile_pool(name="kt", bufs=1) as kt_pool,
            tc.tile_pool(name="vaug", bufs=1) as vaug_pool,
            tc.tile_pool(name="qt", bufs=1) as qt_pool,
            tc.tile_pool(name="sumt", bufs=1) as sumt_pool,
            tc.tile_pool(name="maskp", bufs=1) as mask_pool,
            tc.tile_pool(name="xkvp", bufs=1) as xkv_pool,
            tc.tile_pool(name="wkp", bufs=1) as wk_pool,
            tc.tile_pool(name="misc", bufs=1) as misc_pool,
        ):
            # ---- persistent tiles ----
            KT = [kt_pool.tile([128, T], BF16, tag=f"kt{m}", name=f"kt{m}") for m in range(KD)]
            VA = [vaug_pool.tile([128, VW], BF16, tag=f"va{i}", name=f"va{i}") for i in range(NT)]
            QT = [qt_pool.tile([128, TC], BF16, tag=f"qt{m}", name=f"qt{m}") for m in range(KD)]
            SUMT = [sumt_pool.tile([128, TC], BF16, tag=f"st{m}", name=f"st{m}") for m in range(KD)]
            # mask packed per T-tile pair: [128, 1024] = tiles (2i | 2i+1)
            MSK2 = [mask_pool.tile([128, 2 * TC], BF16, tag=f"mk{i}", name=f"mk{i}") for i in range(NT // 2)]
            xkv_sb = xkv_pool.tile([128, KD * T], BF16, tag="xkv")
            wk_sb = wk_pool.tile([128, KD * D], BF16, tag="wk")
            wo_sb = wk_pool.tile([128, KD * D], BF16, tag="wo")
            bo_sb = misc_pool.tile([1, D], F32, tag="bo")
            bo_bc = misc_pool.tile([128, D], F32, tag="bobc")

            def bulk_dmas():
                # priority order = first-use order: xkv feeds the V phase
                # (starts right after Q proj), wk feeds kproj0 mid-V-phase.
                # wv/masks/wo are issued later (late_dmas) in their use order.
                for c in range(4):
                    for k in range(KD):
                        nc.sync.dma_start(
                            out=xkv_sb[:, k * T + c * 512:k * T + (c + 1) * 512],
                            in_=xkvT[k * 128:(k + 1) * 128, c * 512:(c + 1) * 512])
                for k in range(KD):
                    nc.sync.dma_start(out=wk_sb[:, k * D:(k + 1) * D],
                                      in_=wkT[k * 128:(k + 1) * 128, :])

            def late_dmas():
                for i in range(NT // 2):
                    nc.sync.dma_start(out=MSK2[i][:, 0:TC],
                                      in_=maskT[(2 * i) * 128:(2 * i + 1) * 128, :])
                    nc.sync.dma_start(out=MSK2[i][:, TC:2 * TC],
                                      in_=maskT[(2 * i + 1) * 128:(2 * i + 2) * 128, :])
                for k in range(KD):
                    nc.sync.dma_start(out=wo_sb[:, k * D:(k + 1) * D],
                                      in_=woT[k * 128:(k + 1) * 128, :])
            nc.sync.dma_start(out=bo_sb[:], in_=bo[:])
            nc.gpsimd.partition_broadcast(bo_bc[:], bo_sb[:])

            # ones columns of v_aug (col 64 of each head block)
            for i in range(NT):
                ones_cols = VA[i][:].rearrange("p (h c) -> p h c", c=HD + 1)[:, :, HD:HD + 1]
                nc.vector.memset(ones_cols, 1.0)

            def kproj_chunk(m, c, pool):
                """K.T dq-tile m, T-chunk c (512 cols): 8 matmuls + copy."""
                ps = pool.tile([128, 512], F32, tag="ks", name=f"ks{m}_{c}")
                for k in range(KD):
                    nc.tensor.matmul(
                        ps[:],
                        wk_sb[:, k * D + m * 128:k * D + (m + 1) * 128],
                        xkv_sb[:, k * T + c * 512:k * T + (c + 1) * 512],
                        start=(k == 0),
                        stop=(k == KD - 1),
                    )
                nc.vector.tensor_copy(KT[m][:, c * 512:(c + 1) * 512], ps[:])

            # ---- phase Q: q.T -> QT (bf16) ----
            with (
                tc.tile_pool(name="phq", bufs=1) as phq,
                tc.tile_pool(name="psq", bufs=2, space="PSUM") as psq,
            ):
                wq_sb = phq.tile([128, KD * D], BF16, tag="wq")
                xq_sb = phq.tile([128, KD * TC], BF16, tag="xq")
                for k in range(KD):
                    nc.sync.dma_start(out=xq_sb[:, k * TC:(k + 1) * TC],
                                      in_=xqT[k * 128:(k + 1) * 128, :])
                for mq in range(4):
                    for k in range(KD):
                        nc.sync.dma_start(
                            out=wq_sb[:, k * D + mq * 256:k * D + (mq + 1) * 256],
                            in_=wqT[k * 128:(k + 1) * 128, mq * 256:(mq + 1) * 256])
                bulk_dmas()
                for m in range(KD):
                    ps = psq.tile([128, TC], F32, tag="ps")
                    for k in range(KD):
                        nc.tensor.matmul(
                            ps[:],
                            wq_sb[:, k * D + m * 128:k * D + (m + 1) * 128],
                            xq_sb[:, k * TC:(k + 1) * TC],
                            start=(k == 0),
                            stop=(k == KD - 1),
                        )
                    nc.scalar.copy(QT[m][:], ps[:])

            # ---- phase V (+ KT[0]): ones-augmented V tiles ----
            with (
                tc.tile_pool(name="phv", bufs=1) as phv,
                tc.tile_pool(name="psv", bufs=2, space="PSUM") as psvp,
                tc.tile_pool(name="psk0", bufs=2, space="PSUM") as psk0,
            ):
                wv_sb = phv.tile([128, KD * D], BF16, tag="wv")
                for k in range(KD):
                    nc.sync.dma_start(out=wv_sb[:, k * D:(k + 1) * D],
                                      in_=wvT[k * 128:(k + 1) * 128, :])
                late_dmas()
                for i in range(NT):
                    for dvc in range(2):
                        ps = psvp.tile([128, 512], F32, tag="ps")
                        for k in range(KD):
                            nc.tensor.matmul(
                                ps[:],
                                xkv_sb[:, k * T + i * 128:k * T + (i + 1) * 128],
                                wv_sb[:, k * D + dvc * 512:k * D + (dvc + 1) * 512],
                                start=(k == 0),
                                stop=(k == KD - 1),
                            )
                        dst = (
                            VA[i][:, dvc * 8 * (HD + 1):(dvc + 1) * 8 * (HD + 1)]
                            .rearrange("p (h c) -> p h c", c=HD + 1)[:, :, 0:HD]
                        )
                        src = ps[:].rearrange("p (h c) -> p h c", c=HD)
                        nc.vector.tensor_copy(dst, src)
                    if i % 4 == 3:
                        kproj_chunk(0, i // 4, psk0)

            # ---- attention (head pairs; even/odd QK row-tiled concurrent;
            #      K-proj m=1..7 interleaved) ----
            with (
                tc.tile_pool(name="spool", bufs=1, space="PSUM") as spool,
                tc.tile_pool(name="pvpool", bufs=2, space="PSUM") as pvpool,
                tc.tile_pool(name="kspool", bufs=2, space="PSUM") as kspool,
                tc.tile_pool(name="ptpool", bufs=2) as ptpool,
                tc.tile_pool(name="pt2pool", bufs=3) as pt2pool,
                tc.tile_pool(name="rpool", bufs=1) as rpool,
            ):
                for hp in range(H // 2):
                    ktile = KT[hp]
                    qh_e = QT[hp][0:HD, :]
                    qh_o = QT[hp][HD:128, :]
                    pv_e = pvpool.tile([HD + 1, TC], F32, tag="pv", name=f"pve{hp}")
                    pv_o = pvpool.tile([HD + 1, TC], F32, tag="pv", name=f"pvo{hp}")
                    pts = {}

                    def pv_step(ti):
                        pt2a, pt2b = pts.pop(ti)
                        for j, pt2 in ((0, pt2a), (1, pt2b)):
                            i = 2 * ti + j
                            nc.tensor.matmul(
                                pv_e[:],
                                VA[i][:, (2 * hp) * (HD + 1):(2 * hp + 1) * (HD + 1)],
                                pt2[:, 0:TC],
                                start=(i == 0),
                                stop=(i == NT - 1),
                            )
                            nc.tensor.matmul(
                                pv_o[:],
                                VA[i][:, (2 * hp + 1) * (HD + 1):(2 * hp + 2) * (HD + 1)],
                                pt2[:, TC:2 * TC],
                                start=(i == 0),
                                stop=(i == NT - 1),
                            )

                    for ti in range(NT // 2):
                        # two half tiles, each [even-head | odd-head] scores of
                        # ONE T-tile: the e/o matmul pair shares a buffer (so
                        # the scheduler keeps them adjacent -> PE row groups
                        # 0/64 run concurrently), and exp on half a frees the
                        # next QK pair while exp on half b runs -> ACT stays
                        # dense (no QK-after-exp serialization).
                        s_a = spool.tile([128, 2 * TC], F32, tag="sa", name=f"sa{hp}_{ti}")
                        s_b = spool.tile([128, 2 * TC], F32, tag="sb", name=f"sb{hp}_{ti}")
                        for j, s in ((0, s_a), (1, s_b)):
                            i = 2 * ti + j
                            nc.tensor.matmul(
                                s[:, 0:TC],
                                ktile[0:HD, i * 128:(i + 1) * 128],
                                qh_e,
                                start=True,
                                stop=True,
                            )
                            nc.tensor.matmul(
                                s[:, TC:2 * TC],
                                ktile[HD:128, i * 128:(i + 1) * 128],
                                qh_o,
                                start=True,
                                stop=True,
                            )
                        pt_a = ptpool.tile([128, 2 * TC], BF16, tag="pta", name=f"pta{hp}_{ti}")
                        nc.scalar.activation(
                            pt_a[:], s_a[:], mybir.ActivationFunctionType.Exp,
                            scale=float(EXP_SCALE),
                        )
                        pt_b = ptpool.tile([128, 2 * TC], BF16, tag="ptb", name=f"ptb{hp}_{ti}")
                        nc.scalar.activation(
                            pt_b[:], s_b[:], mybir.ActivationFunctionType.Exp,
                            scale=float(EXP_SCALE),
                        )
                        pt2a = pt2pool.tile([128, 2 * TC], BF16, tag="pt2a", name=f"pt2a{hp}_{ti}")
                        pt2b = pt2pool.tile([128, 2 * TC], BF16, tag="pt2b", name=f"pt2b{hp}_{ti}")
                        # mask for T-tile 2ti is MSK2[ti][:, 0:TC], for 2ti+1
                        # it's [:, TC:2TC]; each applies to both head halves
                        nc.vector.tensor_mul(pt2a[:, 0:TC], pt_a[:, 0:TC], MSK2[ti][:, 0:TC])
                        nc.vector.tensor_mul(pt2a[:, TC:2 * TC], pt_a[:, TC:2 * TC], MSK2[ti][:, 0:TC])
                        nc.vector.tensor_mul(pt2b[:, 0:TC], pt_b[:, 0:TC], MSK2[ti][:, TC:2 * TC])
                        nc.vector.tensor_mul(pt2b[:, TC:2 * TC], pt_b[:, TC:2 * TC], MSK2[ti][:, TC:2 * TC])
                        pts[ti] = (pt2a, pt2b)
                        if ti >= 2:
                            pv_step(ti - 2)
                        # interleaved K-projection: KT[hp+1] built during pair
                        # hp (4 chunks at ti 1,3,5,7)
                        if hp < KD - 1 and ti % 2 == 1:
                            kproj_chunk(hp + 1, ti // 2, kspool)
                    pv_step(NT // 2 - 2)
                    pv_step(NT // 2 - 1)

                    # stash denom (+eps) at partition 0, batch the pair's two
                    # denominators, normalize immediately so SUMT[hp] is final
                    # at pair end (the out-projection chain over m=pairs can
                    # then start without waiting a 4-head recip batch)
                    den2 = rpool.tile([2, TC], F32, tag="den2", name=f"den2_{hp}")
                    for h, pv in ((2 * hp, pv_e), (2 * hp + 1, pv_o)):
                        hb = (h % 2) * HD
                        dtmp = rpool.tile([1, TC], F32, tag="dtmp", name=f"dtmp{h}")
                        nc.vector.tensor_scalar_add(dtmp[:], pv[HD:HD + 1, :], 1e-30)
                        nc.sync.dma_start(out=den2[h % 2:h % 2 + 1, :], in_=dtmp[:])
                        nc.vector.tensor_copy(SUMT[h // 2][hb:hb + HD, :], pv[0:HD, :])
                    rec2 = rpool.tile([2, TC], F32, tag="rec2", name=f"rec2_{hp}")
                    nc.vector.reciprocal_approx_fast(rec2[:], den2[:])
                    for hh in (2 * hp, 2 * hp + 1):
                        hbb = (hh % 2) * HD
                        rtmp = rpool.tile([1, TC], F32, tag="rtmp", name=f"rtmp{hh}")
                        nc.sync.dma_start(out=rtmp[:], in_=rec2[hh % 2:hh % 2 + 1, :])
                        rbc = rpool.tile([128, TC], F32, tag="rbc", name=f"rbc{hh}")
                        nc.gpsimd.partition_broadcast(rbc[:], rtmp[:])
                        sl = SUMT[hh // 2][hbb:hbb + HD, :]
                        nc.vector.tensor_mul(sl, sl, rbc[hbb:hbb + HD, :])

            # ---- out projection: out = summed @ Wo.T + bo ----
            with (
                tc.tile_pool(name="pso", bufs=8, space="PSUM") as pso,
                tc.tile_pool(name="obuf", bufs=4) as obuf,
            ):
                for ttile in range(TC // 128):
                    for oc in range(2):
                        ps = pso.tile([128, 512], F32, tag="ps")
                        for m in range(KD):
                            nc.tensor.matmul(
                                ps[:],
                                SUMT[m][:, ttile * 128:(ttile + 1) * 128],
                                wo_sb[:, m * D + oc * 512:m * D + (oc + 1) * 512],
                                start=(m == 0),
                                stop=(m == KD - 1),
                            )
                        ob = obuf.tile([128, 512], F32, tag="ob")
                        nc.vector.tensor_add(
                            ob[:], ps[:], bo_bc[:, oc * 512:(oc + 1) * 512]
                        )
                        nc.sync.dma_start(
                            out=out[ttile * 128:(ttile + 1) * 128, oc * 512:(oc + 1) * 512],
                            in_=ob[:],
                        )

    nc.compile()
    return nc


_NC_CACHE = None


def get_nc():
    global _NC_CACHE
    if _NC_CACHE is None:
        _NC_CACHE = build_nc()
    return _NC_CACHE


def make_in_maps(inputs_q, inputs_kv, attention_mask, Wq, Wk, Wv, Wo, bo):
    import ml_dtypes

    bf = ml_dtypes.bfloat16
    in_maps = []
    wqT = np.ascontiguousarray(Wq.T).astype(bf)
    wkT = np.ascontiguousarray(Wk.T).astype(bf)
    wvT = np.ascontiguousarray(Wv.T).astype(bf)
    woT = np.ascontiguousarray(Wo.T).astype(bf)
    bo2 = np.ascontiguousarray(bo.reshape(1, D)).astype(np.float32)
    for c in range(NCORES):
        b, tc_i = c // 4, c % 4
        t0 = tc_i * TC
        in_maps.append({
            "xqT": np.ascontiguousarray(inputs_q[b, t0:t0 + TC, :].T).astype(bf),
            "xkvT": np.ascontiguousarray(inputs_kv[b].T).astype(bf),
            "maskT": np.ascontiguousarray(attention_mask[b, t0:t0 + TC, :].T).astype(bf),
            "wqT": wqT, "wkT": wkT, "wvT": wvT, "woT": woT, "bo": bo2,
        })
    return in_maps


def run(in_maps, trace=False, tmpdir=None):
    nc = get_nc()
    return run_bass_kernel_spmd(
        nc, in_maps, core_ids=list(range(NCORES)), trace=trace, tmpdir=tmpdir
    )


def kernel(inputs_q, inputs_kv, attention_mask, Wq, Wk, Wv, Wo, bo):
    in_maps = make_in_maps(
        np.asarray(inputs_q), np.asarray(inputs_kv), np.asarray(attention_mask),
        np.asarray(Wq), np.asarray(Wk), np.asarray(Wv), np.asarray(Wo),
        np.asarray(bo),
    )
    res = run(in_maps)
    out = np.empty((B, T, D), dtype=np.float32)
    for c in range(NCORES):
        b, tc_i = c // 4, c % 4
        out[b, tc_i * TC:(tc_i + 1) * TC, :] = res.results[c]["out"]
    return out


# revision 21
# speedup vs baseline: 1.0988x; 1.0988x over previous
"""Distributed attention kernel for 8 TRN2 NeuronCores (v8, 294us HW).

Sharding: data-parallel over (batch, t-chunk). Core c handles batch c//4,
query rows (c%4)*512 .. +512. Each core computes full K/V for its batch
(duplicated across the 4 cores of a batch group), its own 512-query-row
slice of attention, and the out-projection for those rows. No collectives
(v4 tried a 4-core AllGather for K/V: correct, but the non-Shared 4-rank
gather took 81us plus ~60us of DMA in/out and cannot be hidden behind the
K->QK->exp dependency chain -> slower than recomputing). fp8 DoubleRow
projections (v3, 298us) fail the 2e-2 gate at 6.3e-2: e4m3's ~4%
quantization passes straight to the projection outputs (signal and noise
both grow as sqrt(K)) and the 9-sigma scores amplify it through exp.
All matmul operands are bf16, f32 PSUM.

Optimizations over the 339.5us v2 baseline (each verified on HW traces):
  * QK^T (contraction 64): the per-iteration even/odd head matmuls write
    halves of one shared PSUM tile, so all become schedulable at the same
    event and the Tile scheduler keeps them adjacent in program order;
    their SBUF base partitions 0/64 land in distinct PE row groups, so
    each e/o pair runs CONCURRENTLY (PE array row tiling) -> QK time
    halves (start deltas 3-6ns on HW).
  * Two [128,1024] score tiles per iteration (T-tile granular, [even|odd]
    layout): exp on tile a frees the next QK pair while exp on tile b
    runs -> ACT issues exps back-to-back (no QK-after-exp serialization;
    exp period 2.7us -> ~1.1us).
  * reciprocal_approx_fast + per-pair normalize: SUMT[m] is final at pair
    m's end, so the out-projection's m-chains never wait on a batched
    4-head reciprocal; pso/obuf deepened to 4 buffers.
  * DMA issue order = first-use order (wq/xq, xkv, wk, wv, masks, wo) so
    the V-phase matmuls aren't starved behind weights needed later.

Math: S.T = (K_h @ Q_h.T)/8 per head; P.T = exp(S.T) * M.T (no max-sub:
scores are ~N(0,1)); [summed.T_h ; denom] from a ones-augmented V in one
PV accumulation; normalize by 1/(denom+eps); denom=0 rows give summed=0
-> out = bo, matching the wipe.

Perf structure: K.T projection tiles 1..7 interleaved into the attention
pair loop to fill ACT-bound gaps and keep the PE HAM-warm. PSUM budget:
s_a+s_b (4 banks) + pv_e+pv_o (2) + interleaved kproj (2) = 8.

Engine occupancy at 294us: TensorE ~84% (the bottleneck: 218us of ideal
matmul cycles with this duplicated-K/V sharding), ACT dense through the
156us attention window, DVE ~57%.
"""

import sys

sys.path.insert(0, "/opt/trn_rl_repo")

import numpy as np

import concourse.bass as bass
import concourse.bacc as bacc
import concourse.mybir as mybir
import concourse.tile as tile
from concourse.bass_utils import run_bass_kernel_spmd

F32 = mybir.dt.float32
BF16 = mybir.dt.bfloat16

B, T, D = 2, 2048, 1024
H, HD = 16, 64
TC = 512
NCORES = 8
KD = D // 128   # 8 d-tiles
NT = T // 128   # 16 T-tiles
VW = H * (HD + 1)  # 1040 v_aug width
EXP_SCALE = 1.0 / np.sqrt(HD)


def build_nc():
    nc = bacc.Bacc(
        "TRN2",
        target_bir_lowering=False,
        debug=False,
        enable_asserts=False,
        num_devices=NCORES,
    )

    xqT = nc.dram_tensor("xqT", [D, TC], BF16, kind="ExternalInput").ap()
    xkvT = nc.dram_tensor("xkvT", [D, T], BF16, kind="ExternalInput").ap()
    maskT = nc.dram_tensor("maskT", [T, TC], BF16, kind="ExternalInput").ap()
    wqT = nc.dram_tensor("wqT", [D, D], BF16, kind="ExternalInput").ap()
    wkT = nc.dram_tensor("wkT", [D, D], BF16, kind="ExternalInput").ap()
    wvT = nc.dram_tensor("wvT", [D, D], BF16, kind="ExternalInput").ap()
    woT = nc.dram_tensor("woT", [D, D], BF16, kind="ExternalInput").ap()
    bo = nc.dram_tensor("bo", [1, D], F32, kind="ExternalInput").ap()
    out = nc.dram_tensor("out", [TC, D], F32, kind="ExternalOutput").ap()

    with tile.TileContext(nc) as tc:
        with (
            tc.tile_pool(name="kt", bufs=1) as kt_pool,
            tc.tile_pool(name="vaug", bufs=1) as vaug_pool,
            tc.tile_pool(name="qt", bufs=1) as qt_pool,
            tc.tile_pool(name="sumt", bufs=1) as sumt_pool,
            tc.tile_pool(name="maskp", bufs=1) as mask_pool,
            tc.tile_pool(name="xkvp", bufs=1) as xkv_pool,
            tc.tile_pool(name="wkp", bufs=1) as wk_pool,
            tc.tile_pool(name="misc", bufs=1) as misc_pool,
        ):
            # ---- persistent tiles ----
            KT = [kt_pool.tile([128, T], BF16, tag=f"kt{m}", name=f"kt{m}") for m in range(KD)]
            VA = [vaug_pool.tile([128, VW], BF16, tag=f"va{i}", name=f"va{i}") for i in range(NT)]
            QT = [qt_pool.tile([128, TC], BF16, tag=f"qt{m}", name=f"qt{m}") for m in range(KD)]
            SUMT = [sumt_pool.tile([128, TC], BF16, tag=f"st{m}", name=f"st{m}") for m in range(KD)]
            # mask packed per T-tile pair: [128, 1024] = tiles (2i | 2i+1)
            MSK2 = [mask_pool.tile([128, 2 * TC], BF16, tag=f"mk{i}", name=f"mk{i}") for i in range(NT // 2)]
            xkv_sb = xkv_pool.tile([128, KD * T], BF16, tag="xkv")
            wk_sb = wk_pool.tile([128, KD * D], BF16, tag="wk")
            wo_sb = wk_pool.tile([128, KD * D], BF16, tag="wo")
            bo_sb = misc_pool.tile([1, D], F32, tag="bo")
            bo_bc = misc_pool.tile([128, D], F32, tag="bobc")

            def bulk_dmas():
                # priority order = first-use order: xkv feeds the V phase
                # (starts right after Q proj), wk feeds kproj0 mid-V-phase.
                # wv/masks/wo are issued later (late_dmas) in their use order.
                for k in range(KD):
                    nc.sync.dma_start(out=xkv_sb[:, k * T:(k + 1) * T],
                                      in_=xkvT[k * 128:(k + 1) * 128, :])
                for k in range(KD):
                    nc.sync.dma_start(out=wk_sb[:, k * D:(k + 1) * D],
                                      in_=wkT[k * 128:(k + 1) * 128, :])

            def late_dmas():
                for i in range(NT // 2):
                    nc.sync.dma_start(out=MSK2[i][:, 0:TC],
                                      in_=maskT[(2 * i) * 128:(2 * i + 1) * 128, :])
                    nc.sync.dma_start(out=MSK2[i][:, TC:2 * TC],
                                      in_=maskT[(2 * i + 1) * 128:(2 * i + 2) * 128, :])
                for k in range(KD):
                    nc.sync.dma_start(out=wo_sb[:, k * D:(k + 1) * D],
                                      in_=woT[k * 128:(k + 1) * 128, :])
            nc.sync.dma_start(out=bo_sb[:], in_=bo[:])
            nc.gpsimd.partition_broadcast(bo_bc[:], bo_sb[:])

            # ones columns of v_aug (col 64 of each head block)
            for i in range(NT):
                ones_cols = VA[i][:].rearrange("p (h c) -> p h c", c=HD + 1)[:, :, HD:HD + 1]
                nc.vector.memset(ones_cols, 1.0)

            def kproj_chunk(m, c, pool):
                """K.T dq-tile m, T-chunk c (512 cols): 8 matmuls + copy."""
                ps = pool.tile([128, 512], F32, tag="ks", name=f"ks{m}_{c}")
                for k in range(KD):
                    nc.tensor.matmul(
                        ps[:],
                        wk_sb[:, k * D + m * 128:k * D + (m + 1) * 128],
                        xkv_sb[:, k * T + c * 512:k * T + (c + 1) * 512],
                        start=(k == 0),
                        stop=(k == KD - 1),
                    )
                nc.vector.tensor_copy(KT[m][:, c * 512:(c + 1) * 512], ps[:])

            # ---- phase Q: q.T -> QT (bf16) ----
            with (
                tc.tile_pool(name="phq", bufs=1) as phq,
                tc.tile_pool(name="psq", bufs=2, space="PSUM") as psq,
            ):
                wq_sb = phq.tile([128, KD * D], BF16, tag="wq")
                xq_sb = phq.tile([128, KD * TC], BF16, tag="xq")
                for k in range(KD):
                    nc.sync.dma_start(out=wq_sb[:, k * D:(k + 1) * D],
                                      in_=wqT[k * 128:(k + 1) * 128, :])
                    nc.sync.dma_start(out=xq_sb[:, k * TC:(k + 1) * TC],
                                      in_=xqT[k * 128:(k + 1) * 128, :])
                bulk_dmas()
                for m in range(KD):
                    ps = psq.tile([128, TC], F32, tag="ps")
                    for k in range(KD):
                        nc.tensor.matmul(
                            ps[:],
                            wq_sb[:, k * D + m * 128:k * D + (m + 1) * 128],
                            xq_sb[:, k * TC:(k + 1) * TC],
                            start=(k == 0),
                            stop=(k == KD - 1),
                        )
                    nc.scalar.copy(QT[m][:], ps[:])

            # ---- phase V (+ KT[0]): ones-augmented V tiles ----
            with (
                tc.tile_pool(name="phv", bufs=1) as phv,
                tc.tile_pool(name="psv", bufs=2, space="PSUM") as psvp,
                tc.tile_pool(name="psk0", bufs=2, space="PSUM") as psk0,
            ):
                wv_sb = phv.tile([128, KD * D], BF16, tag="wv")
                for k in range(KD):
                    nc.sync.dma_start(out=wv_sb[:, k * D:(k + 1) * D],
                                      in_=wvT[k * 128:(k + 1) * 128, :])
                late_dmas()
                for i in range(NT):
                    for dvc in range(2):
                        ps = psvp.tile([128, 512], F32, tag="ps")
                        for k in range(KD):
                            nc.tensor.matmul(
                                ps[:],
                                xkv_sb[:, k * T + i * 128:k * T + (i + 1) * 128],
                                wv_sb[:, k * D + dvc * 512:k * D + (dvc + 1) * 512],
                                start=(k == 0),
                                stop=(k == KD - 1),
                            )
                        dst = (
                            VA[i][:, dvc * 8 * (HD + 1):(dvc + 1) * 8 * (HD + 1)]
                            .rearrange("p (h c) -> p h c", c=HD + 1)[:, :, 0:HD]
                        )
                        src = ps[:].rearrange("p (h c) -> p h c", c=HD)
                        nc.vector.tensor_copy(dst, src)
                    if i % 4 == 3:
                        kproj_chunk(0, i // 4, psk0)

            # ---- attention (head pairs; even/odd QK row-tiled concurrent;
            #      K-proj m=1..7 interleaved) ----
            with (
                tc.tile_pool(name="spool", bufs=1, space="PSUM") as spool,
                tc.tile_pool(name="pvpool", bufs=2, space="PSUM") as pvpool,
                tc.tile_pool(name="kspool", bufs=2, space="PSUM") as kspool,
                tc.tile_pool(name="ptpool", bufs=2) as ptpool,
                tc.tile_pool(name="pt2pool", bufs=3) as pt2pool,
                tc.tile_pool(name="rpool", bufs=1) as rpool,
            ):
                for hp in range(H // 2):
                    ktile = KT[hp]
                    qh_e = QT[hp][0:HD, :]
                    qh_o = QT[hp][HD:128, :]
                    pv_e = pvpool.tile([HD + 1, TC], F32, tag="pv", name=f"pve{hp}")
                    pv_o = pvpool.tile([HD + 1, TC], F32, tag="pv", name=f"pvo{hp}")
                    pts = {}

                    def pv_step(ti):
                        pt2a, pt2b = pts.pop(ti)
                        for j, pt2 in ((0, pt2a), (1, pt2b)):
                            i = 2 * ti + j
                            nc.tensor.matmul(
                                pv_e[:],
                                VA[i][:, (2 * hp) * (HD + 1):(2 * hp + 1) * (HD + 1)],
                                pt2[:, 0:TC],
                                start=(i == 0),
                                stop=(i == NT - 1),
                            )
                            nc.tensor.matmul(
                                pv_o[:],
                                VA[i][:, (2 * hp + 1) * (HD + 1):(2 * hp + 2) * (HD + 1)],
                                pt2[:, TC:2 * TC],
                                start=(i == 0),
                                stop=(i == NT - 1),
                            )

                    for ti in range(NT // 2):
                        # two half tiles, each [even-head | odd-head] scores of
                        # ONE T-tile: the e/o matmul pair shares a buffer (so
                        # the scheduler keeps them adjacent -> PE row groups
                        # 0/64 run concurrently), and exp on half a frees the
                        # next QK pair while exp on half b runs -> ACT stays
                        # dense (no QK-after-exp serialization).
                        s_a = spool.tile([128, 2 * TC], F32, tag="sa", name=f"sa{hp}_{ti}")
                        s_b = spool.tile([128, 2 * TC], F32, tag="sb", name=f"sb{hp}_{ti}")
                        for j, s in ((0, s_a), (1, s_b)):
                            i = 2 * ti + j
                            nc.tensor.matmul(
                                s[:, 0:TC],
                                ktile[0:HD, i * 128:(i + 1) * 128],
                                qh_e,
                                start=True,
                                stop=True,
                            )
                            nc.tensor.matmul(
                                s[:, TC:2 * TC],
                                ktile[HD:128, i * 128:(i + 1) * 128],
                                qh_o,
                                start=True,
                                stop=True,
                            )
                        pt_a = ptpool.tile([128, 2 * TC], BF16, tag="pta", name=f"pta{hp}_{ti}")
                        nc.scalar.activation(
                            pt_a[:], s_a[:], mybir.ActivationFunctionType.Exp,
                            scale=float(EXP_SCALE),
                        )
                        pt_b = ptpool.tile([128, 2 * TC], BF16, tag="ptb", name=f"ptb{hp}_{ti}")
                        nc.scalar.activation(
                            pt_b[:], s_b[:], mybir.ActivationFunctionType.Exp,
                            scale=float(EXP_SCALE),
                        )
                        pt2a = pt2pool.tile([128, 2 * TC], BF16, tag="pt2a", name=f"pt2a{hp}_{ti}")
                        pt2b = pt2pool.tile([128, 2 * TC], BF16, tag="pt2b", name=f"pt2b{hp}_{ti}")
                        # mask for T-tile 2ti is MSK2[ti][:, 0:TC], for 2ti+1
                        # it's [:, TC:2TC]; each applies to both head halves
                        nc.vector.tensor_mul(pt2a[:, 0:TC], pt_a[:, 0:TC], MSK2[ti][:, 0:TC])
                        nc.vector.tensor_mul(pt2a[:, TC:2 * TC], pt_a[:, TC:2 * TC], MSK2[ti][:, 0:TC])
                        nc.vector.tensor_mul(pt2b[:, 0:TC], pt_b[:, 0:TC], MSK2[ti][:, TC:2 * TC])
                        nc.vector.tensor_mul(pt2b[:, TC:2 * TC], pt_b[:, TC:2 * TC], MSK2[ti][:, TC:2 * TC])
                        pts[ti] = (pt2a, pt2b)
                        if ti >= 2:
                            pv_step(ti - 2)
                        # interleaved K-projection: KT[hp+1] built during pair
                        # hp (4 chunks at ti 1,3,5,7)
                        if hp < KD - 1 and ti % 2 == 1:
                            kproj_chunk(hp + 1, ti // 2, kspool)
                    pv_step(NT // 2 - 2)
                    pv_step(NT // 2 - 1)

                    # stash denom (+eps) at partition 0, batch the pair's two
                    # denominators, normalize immediately so SUMT[hp] is final
                    # at pair end (the out-projection chain over m=pairs can
                    # then start without waiting a 4-head recip batch)
                    den2 = rpool.tile([2, TC], F32, tag="den2", name=f"den2_{hp}")
                    for h, pv in ((2 * hp, pv_e), (2 * hp + 1, pv_o)):
                        hb = (h % 2) * HD
                        dtmp = rpool.tile([1, TC], F32, tag="dtmp", name=f"dtmp{h}")
                        nc.vector.tensor_scalar_add(dtmp[:], pv[HD:HD + 1, :], 1e-30)
                        nc.sync.dma_start(out=den2[h % 2:h % 2 + 1, :], in_=dtmp[:])
                        nc.vector.tensor_copy(SUMT[h // 2][hb:hb + HD, :], pv[0:HD, :])
                    rec2 = rpool.tile([2, TC], F32, tag="rec2", name=f"rec2_{hp}")
                    nc.vector.reciprocal_approx_fast(rec2[:], den2[:])
                    for hh in (2 * hp, 2 * hp + 1):
                        hbb = (hh % 2) * HD
                        rtmp = rpool.tile([1, TC], F32, tag="rtmp", name=f"rtmp{hh}")
                        nc.sync.dma_start(out=rtmp[:], in_=rec2[hh % 2:hh % 2 + 1, :])
                        rbc = rpool.tile([128, TC], F32, tag="rbc", name=f"rbc{hh}")
                        nc.gpsimd.partition_broadcast(rbc[:], rtmp[:])
                        sl = SUMT[hh // 2][hbb:hbb + HD, :]
                        nc.vector.tensor_mul(sl, sl, rbc[hbb:hbb + HD, :])

            # ---- out projection: out = summed @ Wo.T + bo ----
            with (
                tc.tile_pool(name="pso", bufs=8, space="PSUM") as pso,
                tc.tile_pool(name="obuf", bufs=4) as obuf,
            ):
                for ttile in range(TC // 128):
                    for oc in range(2):
                        ps = pso.tile([128, 512], F32, tag="ps")
                        for m in range(KD):
                            nc.tensor.matmul(
                                ps[:],
                                SUMT[m][:, ttile * 128:(ttile + 1) * 128],
                                wo_sb[:, m * D + oc * 512:m * D + (oc + 1) * 512],
                                start=(m == 0),
                                stop=(m == KD - 1),
                            )
                        ob = obuf.tile([128, 512], F32, tag="ob")
                        nc.vector.tensor_add(
                            ob[:], ps[:], bo_bc[:, oc * 512:(oc + 1) * 512]
                        )
                        nc.sync.dma_start(
                            out=out[ttile * 128:(ttile + 1) * 128, oc * 512:(oc + 1) * 512],
                            in_=ob[:],
                        )

    nc.compile()
    return nc


_NC_CACHE = None


def get_nc():
    global _NC_CACHE
    if _NC_CACHE is None:
        _NC_CACHE = build_nc()
    return _NC_CACHE


def make_in_maps(inputs_q, inputs_kv, attention_mask, Wq, Wk, Wv, Wo, bo):
    import ml_dtypes

    bf = ml_dtypes.bfloat16
    in_maps = []
    wqT = np.ascontiguousarray(Wq.T).astype(bf)
    wkT = np.ascontiguousarray(Wk.T).astype(bf)
    wvT = np.ascontiguousarray(Wv.T).astype(bf)
    woT = np.ascontiguousarray(Wo.T).astype(bf)
    bo2 = np.ascontiguousarray(bo.reshape(1, D)).astype(np.float32)
    for c in range(NCORES):
        b, tc_i = c // 4, c % 4
        t0 = tc_i * TC
        in_maps.append({
            "xqT": np.ascontiguousarray(inputs_q[b, t0:t0 + TC, :].T).astype(bf),
            "xkvT": np.ascontiguousarray(inputs_kv[b].T).astype(bf),
            "maskT": np.ascontiguousarray(attention_mask[b, t0:t0 + TC, :].T).astype(bf),
            "wqT": wqT, "wkT": wkT, "wvT": wvT, "woT": woT, "bo": bo2,
        })
    return in_maps


def run(in_maps, trace=False, tmpdir=None):
    nc = get_nc()
    return run_bass_kernel_spmd(
        nc, in_maps, core_ids=list(range(NCORES)), trace=trace, tmpdir=tmpdir
    )


def kernel(inputs_q, inputs_kv, attention_mask, Wq, Wk, Wv, Wo, bo):
    in_maps = make_in_maps(
        np.asarray(inputs_q), np.asarray(inputs_kv), np.asarray(attention_mask),
        np.asarray(Wq), np.asarray(Wk), np.asarray(Wv), np.asarray(Wo),
        np.asarray(bo),
    )
    res = run(in_maps)
    out = np.empty((B, T, D), dtype=np.float32)
    for c in range(NCORES):
        b, tc_i = c // 4, c % 4
        out[b, tc_i * TC:(tc_i + 1) * TC, :] = res.results[c]["out"]
    return out
